# revision 15
# baseline (speedup 1.0000x reference)
"""CRF NLL kernel for Trainium2 (8 NeuronCores).

Problem: nn_CRF_40278203301966
  emissions [512, 1024, 48] f32, tags [512, 1024] int, mask [512, 1024] bool
  (all ones), transitions [48, 48], start/end transitions [48].
  Output: scalar mean NLL = mean_b(logZ_b - gold_b).

Strategy
--------
The log-partition forward recurrence is rewritten in linear space:

    alpha_t = (P^T alpha_{t-1}) * E_t        P = exp(transitions), E = exp(emissions)

with periodic per-column rescaling (colsum) whose logs telescope into logZ.
P is padded with a ones-column so each matmul also emits the colsum for free.

Sharding: 8 cores = 4 batch groups (128 rows) x 2 sequence halves (512 steps).
Each core runs 4 independent 128-step chunks in parallel (512 matmul columns);
chunk boundaries are stitched with a W=10 warm-up: the transition operator is a
strict Birkhoff contraction (factor <= tanh(0.1) ~ 0.1 per step), so after 10
steps the state direction is correct to ~1e-10 regardless of init.

The host pre-computes exp(emissions) transposed to [slot, tag, column] layout
so the device consumes a single unit-stride stream; the device does the 1024
sequential transition steps (the actual hard part) as 2x138 fp32r matmuls per
core plus DVE emission multiplies.  The gold (numerator) path is a cheap
embarrassingly-parallel gather+sum done on host, as are the tiny final
per-batch log reductions.
"""

import numpy as np
from contextlib import ExitStack

B, S, T = 512, 1024, 48
NCORES = 8
NBG = 4          # batch groups
BG = B // NBG    # 128 rows per group
CHUNKS = 4       # chunks per core
NCOL = CHUNKS * BG   # 512 matmul columns per core
W = 10           # warm-up slots per chunk
KACC = S // (2 * CHUNKS)   # 128 accounted steps per chunk
SLOTS = W + KACC           # 138
RESCALE_SLOTS = list(range(W, W + 121, 8))   # 10,18,...,130
NRESC = len(RESCALE_SLOTS)                   # 16
HALF = NCOL // 2
TP = 64          # emission rows padded 48->64 for partition alignment
M = TP + 1       # matmul output partitions: 0..47 state, 64 colsum

_PROGRAM_CACHE = {}


def _build_program():
    if "nc" in _PROGRAM_CACHE:
        return _PROGRAM_CACHE["nc"]

    import concourse.bacc as bacc
    import concourse.tile as tile
    from concourse import mybir

    f32 = mybir.dt.float32
    f32r = mybir.dt.float32r

    nc = bacc.Bacc("TRN2")
    emis_d = nc.declare_dram_parameter("emis", [SLOTS * TP, NCOL], f32, isOutput=False)
    lhst_d = nc.declare_dram_parameter("lhst", [T, M], f32r, isOutput=False)
    vinit_d = nc.declare_dram_parameter("vinit", [T, NCOL], f32r, isOutput=False)
    final_d = nc.declare_dram_parameter("final", [T, NCOL], f32r, isOutput=True)
    stash_d = nc.declare_dram_parameter("stash", [1, NRESC * NCOL], f32, isOutput=True)

    with tile.TileContext(nc) as tc, ExitStack() as ctx:
        const = ctx.enter_context(tc.tile_pool(name="const", bufs=1))
        epool = ctx.enter_context(tc.tile_pool(name="epool", bufs=6))
        spoolA = ctx.enter_context(tc.tile_pool(name="spoolA", bufs=3))
        spoolB = ctx.enter_context(tc.tile_pool(name="spoolB", bufs=3))
        ppool = ctx.enter_context(tc.tile_pool(name="ppool", bufs=4, space="PSUM"))
        misc = ctx.enter_context(tc.tile_pool(name="misc", bufs=4))

        # Stage lhsT/vinit through a DVE copy so the first matmul waits on a
        # single engine semaphore (PE matmuls take few sync-wait slots).
        lhsT_dma = const.tile([T, M], f32r)
        nc.sync.dma_start(out=lhsT_dma, in_=lhst_d[:, :])
        lhsT = const.tile([T, M], f32r)
        nc.vector.tensor_copy(lhsT, lhsT_dma)
        stash = const.tile([1, NRESC * NCOL], f32)

        vinit_dma = const.tile([T, NCOL], f32r)
        nc.sync.dma_start(out=vinit_dma, in_=vinit_d[:, :])
        spools = (spoolA, spoolB)
        states = []
        for H in (0, 1):
            st = spools[H].tile([T, HALF], f32r)
            nc.vector.tensor_copy(st, vinit_dma[:, H * HALF:(H + 1) * HALF])
            states.append(st)

        etile = None
        for s in range(SLOTS):
            m, rr = divmod(s, 2)
            if rr == 0:
                etile = epool.tile([2 * TP, NCOL], f32)
                nc.sync.dma_start(
                    out=etile, in_=emis_d[2 * TP * m:2 * TP * (m + 1), :]
                )
            for H in (0, 1):
                cs = slice(H * HALF, (H + 1) * HALF)
                ps = ppool.tile([M, HALF], f32)
                nc.tensor.matmul(out=ps, lhsT=lhsT[:, :], rhs=states[H][:, :])
                ns = spools[H].tile([T, HALF], f32r)
                nc.vector.tensor_mul(
                    ns, ps[0:T, :], etile[TP * rr:TP * rr + T, cs]
                )
                if s in RESCALE_SLOTS:
                    k = RESCALE_SLOTS.index(s)
                    rec = misc.tile([1, HALF], f32)
                    nc.vector.reciprocal(rec, ps[TP:TP + 1, :])
                    bc = misc.tile([T, HALF], f32)
                    nc.gpsimd.partition_broadcast(out_ap=bc, in_ap=rec)
                    nc.vector.tensor_mul(ns, ns, bc)
                    nc.vector.tensor_copy(
                        stash[0:1, k * NCOL + H * HALF:k * NCOL + (H + 1) * HALF],
                        ps[TP:TP + 1, :],
                    )
                states[H] = ns

        for H in (0, 1):
            nc.sync.dma_start(
                out=final_d[:, H * HALF:(H + 1) * HALF], in_=states[H]
            )
        nc.sync.dma_start(out=stash_d[:, :], in_=stash)

    nc.compile()
    _PROGRAM_CACHE["nc"] = nc
    return nc


def _host_prep(em, P, startt):
    """Build per-core device input arrays.

    Returns list of 8 arrays [SLOTS*T, NCOL] f32 (core = h*4 + g) plus the
    shared lhsT and vinit arrays.
    """
    # warm-up simulation for the global-start chunk (fp64, b-independent):
    # 10 steps of v <- (P^T v) / 48 from v = 1/48.
    v = np.full(T, 1.0 / T, dtype=np.float64)
    for _ in range(W):
        v = (P.T @ v) / T
    ynorm = v.sum()
    z = P.T @ v                                  # state entering slot W

    expstart = np.exp(startt.astype(np.float64))

    lhst = np.zeros([T, M], np.float32)
    lhst[:, :T] = P.astype(np.float32)
    lhst[:, TP] = 1.0
    vinit = np.full([T, NCOL], 1.0 / T, dtype=np.float32)

    cores = []
    for h in (0, 1):
        tlo = max(0, 4 * h * KACC - W)
        thi = 4 * (h + 1) * KACC
        for g in range(NBG):
            blk = em[g * BG:(g + 1) * BG, tlo:thi, :]          # [128, nt, 48]
            eblk = np.exp(blk).transpose(1, 2, 0)              # [nt, 48, 128]
            dev = np.zeros([SLOTS, TP, NCOL], np.float32)
            for c in range(CHUNKS):
                gc = 4 * h + c                                  # global chunk
                col = slice(c * BG, (c + 1) * BG)
                if gc == 0:
                    dev[:W, :T, col] = 1.0 / T
                    e0 = eblk[0]                               # [48, 128] = t 0
                    dev[W, :T, col] = (
                        e0.astype(np.float64)
                        * (expstart * ynorm / z)[:, None]
                    ).astype(np.float32)
                    dev[W + 1:, :T, col] = eblk[1:KACC]
                else:
                    a = gc * KACC - tlo                        # accounted start
                    dev[:W, :T, col] = eblk[a - W:a]
                    dev[W:, :T, col] = eblk[a:a + KACC]
            cores.append(np.ascontiguousarray(dev.reshape(SLOTS * TP, NCOL)))
    return cores, lhst, vinit


def _host_gold(em, trans, startt, endt, tags, maskf):
    emit = np.take_along_axis(em, tags[:, :, None], axis=2)[..., 0]
    trs = trans[tags[:, :-1], tags[:, 1:]]
    gold = startt[tags[:, 0]] + emit[:, 0]
    gold = gold + ((trs + emit[:, 1:]) * maskf[:, 1:]).sum(axis=1)
    lengths = maskf.astype(np.int64).sum(axis=1) - 1
    last = np.take_along_axis(tags, lengths[:, None], axis=1)[:, 0]
    return gold + endt[last]


def kernel(emissions, transitions, start_transitions, end_transitions, tags, mask):
    from concourse.bass_utils import run_bass_kernel_spmd

    em = np.asarray(emissions, dtype=np.float32)
    trans = np.asarray(transitions, dtype=np.float32)
    startt = np.asarray(start_transitions, dtype=np.float32)
    endt = np.asarray(end_transitions, dtype=np.float32)
    tags_np = np.asarray(tags).astype(np.int64)
    maskf = np.asarray(mask).astype(np.float32)

    P = np.exp(trans.astype(np.float64))

    cores, lhst, vinit = _host_prep(em, P, startt)
    nc = _build_program()
    in_maps = [
        {"emis": cores[i], "lhst": lhst, "vinit": vinit} for i in range(NCORES)
    ]
    res = run_bass_kernel_spmd(nc, in_maps, list(range(NCORES))).results

    expend = np.exp(endt.astype(np.float64))
    logz = np.zeros(B, dtype=np.float64)
    for h in (0, 1):
        for g in range(NBG):
            r = res[h * NBG + g]
            st = r["stash"].reshape(NRESC, NCOL).astype(np.float64)
            fin = r["final"].astype(np.float64)      # [48, 512]
            colsum = fin.sum(axis=0)                 # [512]
            rcols = np.log(st[1:, :]).sum(axis=0) + np.log(colsum)
            logz[g * BG:(g + 1) * BG] += rcols.reshape(CHUNKS, BG).sum(axis=0)
            if h == 1:
                vhat = fin[:, 3 * BG:] / colsum[3 * BG:]
                logz[g * BG:(g + 1) * BG] += np.log(
                    (vhat * expend[:, None]).sum(axis=0)
                )

    gold = _host_gold(em, trans, startt, endt, tags_np, maskf)
    nll = (logz - gold).mean()
    return np.array(nll, dtype=np.float32)


# revision 23
# speedup vs baseline: 2.2130x; 2.2130x over previous
"""CRF NLL kernel for Trainium2 (8 NeuronCores).

Problem: nn_CRF_40278203301966
  emissions [512, 1024, 48] f32, tags [512, 1024] int, mask [512, 1024] bool
  (all ones), transitions [48, 48], start/end transitions [48].
  Output: scalar mean NLL = mean_b(logZ_b - gold_b).

Strategy
--------
The log-partition forward recurrence runs in linear space:

    alpha_t = (P^T alpha_{t-1}) * E_t      P = exp(transitions), E = exp(emissions)

with periodic per-column rescaling whose (exactly stashed) factors telescope
into logZ on the host.

Sharding: 8 cores = 4 batch groups (128 rows) x 2 sequence halves (512 steps).
Per core the 512 steps split into 16 chunks of 32 steps, run in parallel as
matmul columns; each chunk gets a W=8 warm-up (the transition kernel is a
Birkhoff contraction, factor ~0.1/step, so the state direction converges to
~1e-8 regardless of init).  Two chunks stack on the partition dim (rows 0..47
and 64..111) so one [112,512] matmul + one DVE multiply advances 8 chunks;
two such stacks interleave to hide the PE<->DVE dependency latency.

All matmul operands are bf16 (PSUM accumulates fp32); rescale reciprocals run
on the otherwise idle Scalar engine and are applied lazily 2 slots later so
nothing serializes.  The stashed c/rho values make the accounting exact
regardless of rounding.  The gold (numerator) score is a cheap gather+sum done
on the host, as are the final tiny per-batch log reductions.
"""

import numpy as np
from contextlib import ExitStack

import ml_dtypes

BF16 = ml_dtypes.bfloat16

B, S, T = 512, 1024, 48
NCORES = 8
NBG = 4            # batch groups
BG = B // NBG      # 128 rows per group
NP = 112           # partitions: rows 0..47 block A, 64..111 block B
BLK = 64           # block stride
C = 16             # chunks per core
LEN = S // 2 // C  # 32 accounted steps per chunk
W = 8              # warm-up slots
SLOTS = W + LEN    # 40
G = 2              # independent stacks
WCOL = 512         # columns per stack (4 column-chunks x 128 batch)
QC = WCOL // BG    # 4 column-chunks per stack
RESCALES = [8, 16, 24, 32]
NR = len(RESCALES)
APPLY_D = 2        # rescale applied APPLY_D slots later
STASH_ROWS = 2 + 2 * NR        # c_A,c_B + (rho_A,rho_B) per rescale
STASHW = G * STASH_ROWS * WCOL

_PROGRAM_CACHE = {}


def _build_program():
    if "nc" in _PROGRAM_CACHE:
        return _PROGRAM_CACHE["nc"]

    import concourse.bacc as bacc
    import concourse.tile as tile
    from concourse import mybir

    f32 = mybir.dt.float32
    bf16 = mybir.dt.bfloat16

    nc = bacc.Bacc("TRN2")
    emis_d = nc.declare_dram_parameter(
        "emis", [G * SLOTS * NP, WCOL], bf16, isOutput=False
    )
    lhst_d = nc.declare_dram_parameter("lhst", [NP, NP], bf16, isOutput=False)
    ones_d = nc.declare_dram_parameter("ones", [NP, 33], bf16, isOutput=False)
    vinit_d = nc.declare_dram_parameter("vinit", [NP, G * WCOL], bf16, isOutput=False)
    final_d = nc.declare_dram_parameter("final", [NP, G * WCOL], bf16, isOutput=True)
    stash_d = nc.declare_dram_parameter("stash", [1, STASHW], f32, isOutput=True)

    with tile.TileContext(nc) as tc, ExitStack() as ctx:
        const = ctx.enter_context(tc.tile_pool(name="const", bufs=1))
        epool = ctx.enter_context(tc.tile_pool(name="epool", bufs=8))
        spool = [
            ctx.enter_context(tc.tile_pool(name=f"spool{g}", bufs=3))
            for g in range(G)
        ]
        ppool = ctx.enter_context(tc.tile_pool(name="ppool", bufs=4, space="PSUM"))
        cpool = ctx.enter_context(tc.tile_pool(name="cpool", bufs=2, space="PSUM"))
        misc = ctx.enter_context(tc.tile_pool(name="misc", bufs=4))
        bcpool = ctx.enter_context(tc.tile_pool(name="bcpool", bufs=4))

        # Stage DMA'd params through a DVE copy so consumers wait on one sem.
        lhsT_dma = const.tile([NP, NP], bf16)
        nc.sync.dma_start(out=lhsT_dma, in_=lhst_d[:, :])
        lhsT = const.tile([NP, NP], bf16)
        nc.vector.tensor_copy(lhsT, lhsT_dma)
        onesT_dma = const.tile([NP, 33], bf16)
        nc.sync.dma_start(out=onesT_dma, in_=ones_d[:, :])
        onesT = const.tile([NP, 33], bf16)
        nc.vector.tensor_copy(onesT, onesT_dma)
        vinit_dma = const.tile([NP, G * WCOL], bf16)
        nc.sync.dma_start(out=vinit_dma, in_=vinit_d[:, :])

        stash = const.tile([1, STASHW], f32)

        states = []
        for g in range(G):
            st = spool[g].tile([NP, WCOL], bf16)
            nc.vector.tensor_copy(st, vinit_dma[:, g * WCOL:(g + 1) * WCOL])
            states.append(st)

        pending_bc = [dict() for _ in range(G)]  # slot -> bc tile

        for s in range(SLOTS):
            for g in range(G):
                row0 = (g * SLOTS + s) * NP
                et = epool.tile([NP, WCOL], bf16)
                nc.sync.dma_start(out=et, in_=emis_d[row0:row0 + NP, :])

                ps = ppool.tile([NP, WCOL], f32)
                nc.tensor.matmul(out=ps, lhsT=lhsT[:, :], rhs=states[g][:, :])

                if s in RESCALES:
                    k = RESCALES.index(s)
                    soff = (g * STASH_ROWS) * WCOL
                    ps2 = cpool.tile([33, WCOL], f32)
                    nc.tensor.matmul(out=ps2, lhsT=onesT[:, :], rhs=states[g][:, :])
                    if k == 0:  # boundary: stash measured colsums
                        nc.vector.tensor_copy(
                            stash[0:1, soff:soff + WCOL], ps2[0:1, :]
                        )
                        nc.vector.tensor_copy(
                            stash[0:1, soff + WCOL:soff + 2 * WCOL], ps2[32:33, :]
                        )
                    recA = misc.tile([1, WCOL], f32)
                    nc.vector.reciprocal_approx_fast(out=recA, in_=ps2[0:1, :])
                    recB = misc.tile([1, WCOL], f32)
                    nc.vector.reciprocal_approx_fast(out=recB, in_=ps2[32:33, :])
                    roff = soff + (2 + 2 * k) * WCOL
                    nc.vector.tensor_copy(stash[0:1, roff:roff + WCOL], recA)
                    nc.vector.tensor_copy(
                        stash[0:1, roff + WCOL:roff + 2 * WCOL], recB
                    )
                    # partition_broadcast writes garbage when its output AP
                    # starts at a nonzero partition, and TensorTensor operands
                    # must share a start partition — so broadcast B at offset
                    # 0 into a scratch tile and DMA-shift it to rows 64..111.
                    bc = bcpool.tile([NP, WCOL], f32, tag="bc")
                    nc.gpsimd.partition_broadcast(out_ap=bc[0:T, :], in_ap=recA)
                    scr = bcpool.tile([T, WCOL], f32, tag="scr")
                    nc.gpsimd.partition_broadcast(out_ap=scr[0:T, :], in_ap=recB)
                    nc.sync.dma_start(out=bc[BLK:NP, :], in_=scr[0:T, :])
                    pending_bc[g][s + APPLY_D] = bc

                ns = spool[g].tile([NP, WCOL], bf16)
                nc.vector.tensor_mul(ns, ps[0:NP, :], et)
                bc = pending_bc[g].pop(s, None)
                if bc is not None:
                    nc.vector.tensor_mul(ns[0:T, :], ns[0:T, :], bc[0:T, :])
                    nc.vector.tensor_mul(
                        ns[BLK:NP, :], ns[BLK:NP, :], bc[BLK:NP, :]
                    )
                states[g] = ns

        for g in range(G):
            nc.sync.dma_start(
                out=final_d[:, g * WCOL:(g + 1) * WCOL], in_=states[g]
            )
        nc.sync.dma_start(out=stash_d[:, :], in_=stash)

    nc.compile()
    _PROGRAM_CACHE["nc"] = nc
    return nc


def _chunk_map(c):
    """chunk index (0..15) -> (stack, rowblock, colchunk)."""
    s0, cc = divmod(c, 8)
    rb, q = divmod(cc, 4)
    return s0, rb, q


def _host_prep(em, P, startt):
    """Build per-core device input arrays.

    Returns (cores, lhst, ones, vinit): cores is a list of 8 bf16 arrays
    [G*SLOTS*NP, WCOL] (core = h*4 + g).
    """
    # warm-up simulation for the global-start chunk (fp64, b-independent):
    # W steps of v <- (P^T v) / 48 from v = 1/48.
    v = np.full(T, 1.0 / T, dtype=np.float64)
    for _ in range(W):
        v = (P.T @ v) / T
    ynorm = v.sum()
    z = P.T @ v

    expstart = np.exp(startt.astype(np.float64))

    lhst = np.zeros([NP, NP], np.float32)
    lhst[0:T, 0:T] = P.astype(np.float32)
    lhst[BLK:BLK + T, BLK:BLK + T] = P.astype(np.float32)
    ones = np.zeros([NP, 33], np.float32)
    ones[0:T, 0] = 1.0
    ones[BLK:BLK + T, 32] = 1.0
    vinit = np.zeros([NP, G * WCOL], np.float32)
    vinit[0:T] = 1.0 / T
    vinit[BLK:BLK + T] = 1.0 / T

    cores = []
    for h in (0, 1):
        for g in range(NBG):
            blk = em[g * BG:(g + 1) * BG, 512 * h:512 * (h + 1), :]
            eblk = np.exp(blk, dtype=np.float32).transpose(1, 2, 0)  # [512,48,128]
            dev = np.zeros([G, SLOTS, NP, WCOL], np.float32)
            for c in range(C):
                gc = C * h + c
                s0, rb, q = _chunk_map(c)
                rows = slice(BLK * rb, BLK * rb + T)
                cols = slice(q * BG, (q + 1) * BG)
                a = LEN * c  # accounted start within this core's eblk
                if gc == 0:
                    dev[s0, :W, rows, cols] = 1.0 / T
                    e0 = eblk[0]
                    dev[s0, W, rows, cols] = (
                        e0.astype(np.float64) * (expstart * ynorm / z)[:, None]
                    ).astype(np.float32)
                    dev[s0, W + 1:, rows, cols] = eblk[1:LEN]
                elif c == 0:
                    # warm-up crosses the core boundary: read from prev half
                    pe = np.exp(
                        em[g * BG:(g + 1) * BG, 512 * h - W:512 * h, :],
                        dtype=np.float32,
                    ).transpose(1, 2, 0)
                    dev[s0, :W, rows, cols] = pe
                    dev[s0, W:, rows, cols] = eblk[:LEN]
                else:
                    dev[s0, :W, rows, cols] = eblk[a - W:a]
                    dev[s0, W:, rows, cols] = eblk[a:a + LEN]
            cores.append(
                np.ascontiguousarray(
                    dev.reshape(G * SLOTS * NP, WCOL).astype(BF16)
                )
            )
    return cores, lhst.astype(BF16), ones.astype(BF16), vinit.astype(BF16)


def _host_gold(em, trans, startt, endt, tags, maskf):
    emit = np.take_along_axis(em, tags[:, :, None], axis=2)[..., 0]
    trs = trans[tags[:, :-1], tags[:, 1:]]
    gold = startt[tags[:, 0]] + emit[:, 0]
    gold = gold + ((trs + emit[:, 1:]) * maskf[:, 1:]).sum(axis=1)
    lengths = maskf.astype(np.int64).sum(axis=1) - 1
    last = np.take_along_axis(tags, lengths[:, None], axis=1)[:, 0]
    return gold + endt[last]


def _stitch(results, endt):
    """Combine device outputs into per-batch logZ [B] (fp64)."""
    expend = np.exp(endt.astype(np.float64))
    logz = np.zeros(B, dtype=np.float64)
    for h in (0, 1):
        for g in range(NBG):
            r = results[h * NBG + g]
            st = r["stash"].reshape(G, STASH_ROWS, WCOL).astype(np.float64)
            fin = r["final"].astype(np.float64)  # [NP, G*WCOL]
            for c in range(C):
                gc = C * h + c
                s0, rb, q = _chunk_map(c)
                rows = slice(BLK * rb, BLK * rb + T)
                cols = slice(s0 * WCOL + q * BG, s0 * WCOL + (q + 1) * BG)
                scols = slice(q * BG, (q + 1) * BG)
                fb = fin[rows, cols]                      # [48, 128]
                colsum = fb.sum(axis=0)
                cb = st[s0, rb, scols]                    # boundary colsum
                rhos = st[s0, 2 + rb::2, scols][:NR]      # [NR, 128]
                r_c = np.log(colsum) - np.log(cb) - np.log(rhos).sum(axis=0)
                logz[g * BG:(g + 1) * BG] += r_c
                if gc == 2 * C - 1:  # global last chunk: end-transitions term
                    vhat = fb / colsum
                    logz[g * BG:(g + 1) * BG] += np.log(
                        (vhat * expend[:, None]).sum(axis=0)
                    )
    return logz


def kernel(emissions, transitions, start_transitions, end_transitions, tags, mask):
    from concourse.bass_utils import run_bass_kernel_spmd

    em = np.asarray(emissions, dtype=np.float32)
    trans = np.asarray(transitions, dtype=np.float32)
    startt = np.asarray(start_transitions, dtype=np.float32)
    endt = np.asarray(end_transitions, dtype=np.float32)
    tags_np = np.asarray(tags).astype(np.int64)
    maskf = np.asarray(mask).astype(np.float32)

    P = np.exp(trans.astype(np.float64))
    cores, lhst, ones, vinit = _host_prep(em, P, startt)
    nc = _build_program()
    in_maps = [
        {"emis": cores[i], "lhst": lhst, "ones": ones, "vinit": vinit}
        for i in range(NCORES)
    ]
    res = run_bass_kernel_spmd(nc, in_maps, list(range(NCORES))).results

    logz = _stitch(res, endt)
    gold = _host_gold(em, trans, startt, endt, tags_np, maskf)
    nll = (logz - gold).mean()
    return np.array(nll, dtype=np.float32)


# revision 29
# speedup vs baseline: 2.6547x; 1.1996x over previous
"""CRF NLL kernel for Trainium2 (8 NeuronCores).

Problem: nn_CRF_40278203301966
  emissions [512, 1024, 48] f32, tags [512, 1024] int, mask [512, 1024] bool
  (all ones), transitions [48, 48], start/end transitions [48].
  Output: scalar mean NLL = mean_b(logZ_b - gold_b).

Strategy
--------
The log-partition forward recurrence runs in linear space:

    alpha_t = (P^T alpha_{t-1}) * E_t      P = exp(transitions), E = exp(emissions)

with periodic per-column rescaling whose (exactly stashed) factors telescope
into logZ on the host.

Sharding: 8 cores = 4 batch groups (128 rows) x 2 sequence halves (512 steps).
Per core the 512 steps split into 16 chunks of 32 steps, run in parallel as
matmul columns; each chunk gets a W=8 warm-up (the transition kernel is a
Birkhoff contraction, factor ~0.1/step, so the state direction converges to
~1e-8 regardless of init).  Two chunks stack on the partition dim (rows 0..47
and 64..111) so one [112,512] matmul + one DVE multiply advances 8 chunks;
two such stacks interleave to hide the PE<->DVE dependency latency.

All matmul operands are bf16 (PSUM accumulates fp32); rescale reciprocals run
on the otherwise idle Scalar engine and are applied lazily 2 slots later so
nothing serializes.  The stashed c/rho values make the accounting exact
regardless of rounding.  The gold (numerator) score is a cheap gather+sum done
on the host, as are the final tiny per-batch log reductions.
"""

import numpy as np
from contextlib import ExitStack

import ml_dtypes

BF16 = ml_dtypes.bfloat16

B, S, T = 512, 1024, 48
NCORES = 8
NBG = 4            # batch groups
BG = B // NBG      # 128 rows per group
NP = 112           # partitions: rows 0..47 block A, 64..111 block B
BLK = 64           # block stride
C = 16             # chunks per core
LEN = S // 2 // C  # 32 accounted steps per chunk
W = 6              # warm-up slots
SLOTS = W + LEN    # 38
G = 2              # independent stacks
WCOL = 512         # columns per stack (4 column-chunks x 128 batch)
QC = WCOL // BG    # 4 column-chunks per stack
RESCALES = [6, 14, 22, 30]
NR = len(RESCALES)
APPLY_D = 2        # rescale applied APPLY_D slots later
STASH_ROWS = 2 + 2 * NR        # c_A,c_B + (rho_A,rho_B) per rescale
STASHW = G * STASH_ROWS * WCOL

_PROGRAM_CACHE = {}


def _build_program():
    if "nc" in _PROGRAM_CACHE:
        return _PROGRAM_CACHE["nc"]

    import concourse.bacc as bacc
    import concourse.tile as tile
    from concourse import mybir

    f32 = mybir.dt.float32
    bf16 = mybir.dt.bfloat16

    nc = bacc.Bacc("TRN2")
    emis_d = nc.declare_dram_parameter(
        "emis", [G * SLOTS * NP, WCOL], bf16, isOutput=False
    )
    lhst_d = nc.declare_dram_parameter("lhst", [NP, NP], bf16, isOutput=False)
    ones_d = nc.declare_dram_parameter("ones", [NP, NP], bf16, isOutput=False)
    vinit_d = nc.declare_dram_parameter("vinit", [NP, G * WCOL], bf16, isOutput=False)
    final_d = nc.declare_dram_parameter("final", [NP, G * WCOL], bf16, isOutput=True)
    stash_d = nc.declare_dram_parameter("stash", [1, STASHW], f32, isOutput=True)

    with tile.TileContext(nc) as tc, ExitStack() as ctx:
        const = ctx.enter_context(tc.tile_pool(name="const", bufs=1))
        epool = ctx.enter_context(tc.tile_pool(name="epool", bufs=8))
        spool = [
            ctx.enter_context(tc.tile_pool(name=f"spool{g}", bufs=3))
            for g in range(G)
        ]
        ppool = ctx.enter_context(tc.tile_pool(name="ppool", bufs=4, space="PSUM"))
        cpool = ctx.enter_context(tc.tile_pool(name="cpool", bufs=2, space="PSUM"))
        misc = ctx.enter_context(tc.tile_pool(name="misc", bufs=4))
        bcpool = ctx.enter_context(tc.tile_pool(name="bcpool", bufs=4))

        # Stage DMA'd params through a DVE copy so consumers wait on one sem.
        lhsT_dma = const.tile([NP, NP], bf16)
        nc.sync.dma_start(out=lhsT_dma, in_=lhst_d[:, :])
        lhsT = const.tile([NP, NP], bf16)
        nc.vector.tensor_copy(lhsT, lhsT_dma)
        onesT_dma = const.tile([NP, NP], bf16)
        nc.sync.dma_start(out=onesT_dma, in_=ones_d[:, :])
        onesT = const.tile([NP, NP], bf16)
        nc.vector.tensor_copy(onesT, onesT_dma)
        vinit_dma = const.tile([NP, G * WCOL], bf16)
        nc.sync.dma_start(out=vinit_dma, in_=vinit_d[:, :])

        stash = const.tile([1, STASHW], f32)

        states = []
        for g in range(G):
            st = spool[g].tile([NP, WCOL], bf16)
            nc.vector.tensor_copy(st, vinit_dma[:, g * WCOL:(g + 1) * WCOL])
            states.append(st)

        pending_bc = [dict() for _ in range(G)]  # slot -> bc tile

        for s in range(SLOTS):
            for g in range(G):
                row0 = (g * SLOTS + s) * NP
                et = epool.tile([NP, WCOL], bf16)
                nc.sync.dma_start(out=et, in_=emis_d[row0:row0 + NP, :])

                ps = ppool.tile([NP, WCOL], f32)
                nc.tensor.matmul(out=ps, lhsT=lhsT[:, :], rhs=states[g][:, :])

                if s in RESCALES:
                    # The ones-matmul broadcasts each block's colsum to every
                    # row of that block (lhsT col j has ones over the rows of
                    # j's block), so one reciprocal over [NP, WCOL] yields the
                    # full division tile — no partition_broadcast needed.
                    k = RESCALES.index(s)
                    soff = (g * STASH_ROWS) * WCOL
                    ps2 = cpool.tile([NP, WCOL], f32)
                    nc.tensor.matmul(out=ps2, lhsT=onesT[:, :], rhs=states[g][:, :])
                    if k == 0:  # boundary: stash measured colsums (on ACT)
                        nc.scalar.copy(stash[0:1, soff:soff + WCOL], ps2[0:1, :])
                        nc.scalar.copy(
                            stash[0:1, soff + WCOL:soff + 2 * WCOL],
                            ps2[BLK:BLK + 1, :],
                        )
                    bc = bcpool.tile([NP, WCOL], f32, tag="bc")
                    nc.vector.reciprocal_approx_fast(out=bc, in_=ps2[0:NP, :])
                    roff = soff + (2 + 2 * k) * WCOL
                    nc.gpsimd.tensor_copy(stash[0:1, roff:roff + WCOL], bc[0:1, :])
                    nc.gpsimd.tensor_copy(
                        stash[0:1, roff + WCOL:roff + 2 * WCOL], bc[BLK:BLK + 1, :]
                    )
                    pending_bc[g][s + APPLY_D] = bc

                ns = spool[g].tile([NP, WCOL], bf16)
                nc.vector.tensor_mul(ns, ps[0:NP, :], et)
                bc = pending_bc[g].pop(s, None)
                if bc is not None:
                    nc.vector.tensor_mul(ns, ns, bc)
                states[g] = ns

        for g in range(G):
            nc.sync.dma_start(
                out=final_d[:, g * WCOL:(g + 1) * WCOL], in_=states[g]
            )
        nc.sync.dma_start(out=stash_d[:, :], in_=stash)

    nc.compile()
    _PROGRAM_CACHE["nc"] = nc
    return nc


def _chunk_map(c):
    """chunk index (0..15) -> (stack, rowblock, colchunk)."""
    s0, cc = divmod(c, 8)
    rb, q = divmod(cc, 4)
    return s0, rb, q


def _host_prep(em, P, startt):
    """Build per-core device input arrays.

    Returns (cores, lhst, ones, vinit): cores is a list of 8 bf16 arrays
    [G*SLOTS*NP, WCOL] (core = h*4 + g).
    """
    # warm-up simulation for the global-start chunk (fp64, b-independent):
    # W steps of v <- (P^T v) / 48 from v = 1/48.
    v = np.full(T, 1.0 / T, dtype=np.float64)
    for _ in range(W):
        v = (P.T @ v) / T
    ynorm = v.sum()
    z = P.T @ v

    expstart = np.exp(startt.astype(np.float64))

    lhst = np.zeros([NP, NP], np.float32)
    lhst[0:T, 0:T] = P.astype(np.float32)
    lhst[BLK:BLK + T, BLK:BLK + T] = P.astype(np.float32)
    # ones-matmul col j sums the block that out-row j divides: cols 0..47
    # sum block A; cols 48..111 sum block B (48..63 only keeps recip finite).
    ones = np.zeros([NP, NP], np.float32)
    ones[0:T, 0:T] = 1.0
    ones[BLK:BLK + T, T:NP] = 1.0
    vinit = np.zeros([NP, G * WCOL], np.float32)
    vinit[0:T] = 1.0 / T
    vinit[BLK:BLK + T] = 1.0 / T

    cores = []
    for h in (0, 1):
        for g in range(NBG):
            blk = em[g * BG:(g + 1) * BG, 512 * h:512 * (h + 1), :]
            eblk = np.exp(blk, dtype=np.float32).transpose(1, 2, 0)  # [512,48,128]
            dev = np.zeros([G, SLOTS, NP, WCOL], np.float32)
            for c in range(C):
                gc = C * h + c
                s0, rb, q = _chunk_map(c)
                rows = slice(BLK * rb, BLK * rb + T)
                cols = slice(q * BG, (q + 1) * BG)
                a = LEN * c  # accounted start within this core's eblk
                if gc == 0:
                    dev[s0, :W, rows, cols] = 1.0 / T
                    e0 = eblk[0]
                    dev[s0, W, rows, cols] = (
                        e0.astype(np.float64) * (expstart * ynorm / z)[:, None]
                    ).astype(np.float32)
                    dev[s0, W + 1:, rows, cols] = eblk[1:LEN]
                elif c == 0:
                    # warm-up crosses the core boundary: read from prev half
                    pe = np.exp(
                        em[g * BG:(g + 1) * BG, 512 * h - W:512 * h, :],
                        dtype=np.float32,
                    ).transpose(1, 2, 0)
                    dev[s0, :W, rows, cols] = pe
                    dev[s0, W:, rows, cols] = eblk[:LEN]
                else:
                    dev[s0, :W, rows, cols] = eblk[a - W:a]
                    dev[s0, W:, rows, cols] = eblk[a:a + LEN]
            cores.append(
                np.ascontiguousarray(
                    dev.reshape(G * SLOTS * NP, WCOL).astype(BF16)
                )
            )
    return cores, lhst.astype(BF16), ones.astype(BF16), vinit.astype(BF16)


def _host_gold(em, trans, startt, endt, tags, maskf):
    emit = np.take_along_axis(em, tags[:, :, None], axis=2)[..., 0]
    trs = trans[tags[:, :-1], tags[:, 1:]]
    gold = startt[tags[:, 0]] + emit[:, 0]
    gold = gold + ((trs + emit[:, 1:]) * maskf[:, 1:]).sum(axis=1)
    lengths = maskf.astype(np.int64).sum(axis=1) - 1
    last = np.take_along_axis(tags, lengths[:, None], axis=1)[:, 0]
    return gold + endt[last]


def _stitch(results, endt):
    """Combine device outputs into per-batch logZ [B] (fp64)."""
    expend = np.exp(endt.astype(np.float64))
    logz = np.zeros(B, dtype=np.float64)
    for h in (0, 1):
        for g in range(NBG):
            r = results[h * NBG + g]
            st = r["stash"].reshape(G, STASH_ROWS, WCOL).astype(np.float64)
            fin = r["final"].astype(np.float64)  # [NP, G*WCOL]
            for c in range(C):
                gc = C * h + c
                s0, rb, q = _chunk_map(c)
                rows = slice(BLK * rb, BLK * rb + T)
                cols = slice(s0 * WCOL + q * BG, s0 * WCOL + (q + 1) * BG)
                scols = slice(q * BG, (q + 1) * BG)
                fb = fin[rows, cols]                      # [48, 128]
                colsum = fb.sum(axis=0)
                cb = st[s0, rb, scols]                    # boundary colsum
                rhos = st[s0, 2 + rb::2, scols][:NR]      # [NR, 128]
                r_c = np.log(colsum) - np.log(cb) - np.log(rhos).sum(axis=0)
                logz[g * BG:(g + 1) * BG] += r_c
                if gc == 2 * C - 1:  # global last chunk: end-transitions term
                    vhat = fb / colsum
                    logz[g * BG:(g + 1) * BG] += np.log(
                        (vhat * expend[:, None]).sum(axis=0)
                    )
    return logz


def kernel(emissions, transitions, start_transitions, end_transitions, tags, mask):
    from concourse.bass_utils import run_bass_kernel_spmd

    em = np.asarray(emissions, dtype=np.float32)
    trans = np.asarray(transitions, dtype=np.float32)
    startt = np.asarray(start_transitions, dtype=np.float32)
    endt = np.asarray(end_transitions, dtype=np.float32)
    tags_np = np.asarray(tags).astype(np.int64)
    maskf = np.asarray(mask).astype(np.float32)

    P = np.exp(trans.astype(np.float64))
    cores, lhst, ones, vinit = _host_prep(em, P, startt)
    nc = _build_program()
    in_maps = [
        {"emis": cores[i], "lhst": lhst, "ones": ones, "vinit": vinit}
        for i in range(NCORES)
    ]
    res = run_bass_kernel_spmd(nc, in_maps, list(range(NCORES))).results

    logz = _stitch(res, endt)
    gold = _host_gold(em, trans, startt, endt, tags_np, maskf)
    nll = (logz - gold).mean()
    return np.array(nll, dtype=np.float32)
